# revision 21
# baseline (speedup 1.0000x reference)
"""Trainium2 Bass kernel for nn_AttentionBlock (GN + self-attn + cross-attn + FFN).

Sharding: data-parallel over batch B=8 -> one batch element per NeuronCore.
Per-core layout: activations as [C(partitions), L(free)] "conv" layout.
Attention computed with transposed scores S^T[m, l]; softmax sums come from an
augmented-V matmul (extra ones column -> Z lands in psum partition 64), so no
cross-partition reductions or transposes are needed. Row-softmax max-subtraction
is skipped (logits are provably < 2 for this block's scale).
Matmuls run in float32r (rounded fp32, full PE speed at N>=256); attention
probabilities / V / cross-attention / FFN-hidden run in bf16.
"""
import numpy as np

# ---- problem constants (hardcoded per contract) ----
B, C, H, W = 8, 512, 32, 32
L = H * W                       # 1024
NH, HD = 8, 64
CT = C // 128                   # 4 channel tiles
LT = L // 128                   # 8 l/m tiles
NCH = 2                         # l chunks of 512
CH = L // NCH                   # 512
CTX = 768
S = 77
SP = 128                        # padded context tokens
KTC = CTX // 128                # 6
FF = 4 * C                      # 2048
FT = FF // 128                  # 16
G = 32                          # groups
EPS = 1e-5
SCALE = HD ** -0.5

_CACHE = {}


def _build(gelu_identity=False, stop_after=None, repeat=1):
    import concourse.mybir as mybir
    import concourse.tile as tile
    from concourse import bacc

    f32 = mybir.dt.float32
    f32r = mybir.dt.float32r
    bf16 = mybir.dt.bfloat16
    Exp = mybir.ActivationFunctionType.Exp
    Gelu = (mybir.ActivationFunctionType.Identity if gelu_identity
            else mybir.ActivationFunctionType.Gelu)
    Sqrt = mybir.ActivationFunctionType.Sqrt
    Square = mybir.ActivationFunctionType.Square
    add = mybir.AluOpType.add
    mult = mybir.AluOpType.mult
    AX = mybir.AxisListType.X

    nc = bacc.Bacc("TRN2", target_bir_lowering=False, debug=False, num_devices=8)

    def din(name, shape, dt=f32r):
        return nc.dram_tensor(name, shape, dt, kind="ExternalInput").ap()

    x_d = din("x", [128, CT, L], f32)
    ctxT_d = din("ctxT", [128, KTC, SP], bf16)
    qkvwT_d = din("qkv_wT", [128, CT, 3 * C])
    sapT_d = din("sa_proj_wT", [128, CT, C])
    qwT_d = din("q_wT", [128, CT, C])
    kwT_d = din("k_wT", [128, KTC, C], bf16)
    vwT_d = din("v_wT", [128, KTC, C], bf16)
    capT_d = din("ca_proj_wT", [128, CT, C], bf16)
    w1T_d = din("w1T", [128, CT, FF])
    w2T_d = din("w2T", [128, FT, C], bf16)
    mask_d = din("gn_mask", [128, CT, G], f32)
    maskT_d = din("gn_maskT", [G, C], f32)
    gn1g_d = din("gn1g", [128, CT], f32)
    gn1b_d = din("gn1b", [128, CT], f32)
    gn2g_d = din("gn2g", [128, CT], f32)
    gn2b_d = din("gn2b", [128, CT], f32)
    qkb_d = din("qkb", [128, 2 * CT], f32)     # qkv_b for q,k in conv layout
    vb_row_d = din("vb_row", [1, C], f32)      # qkv_b v-part as a row
    sapb_d = din("sapb", [128, CT], f32)
    qb_d = din("qb", [128, CT], f32)
    kb_d = din("kb", [128, CT], f32)
    vb2_row_d = din("vb2_row", [1, C], f32)
    capb_d = din("capb", [128, CT], f32)
    b1_d = din("b1", [128, FT], f32)
    b2_d = din("b2", [128, CT], f32)
    smask_d = din("smask", [128, 1], f32)      # context token validity column

    out_d = nc.dram_tensor("out", [128, CT, L], f32, kind="ExternalOutput").ap()

    dma = nc.sync.dma_start

    class _Stop(Exception):
        pass

    with tile.TileContext(nc) as tc:
        _stack = []

        def apool(**kw):
            p = tc.alloc_tile_pool(**kw)
            _stack.append(p)
            return p

        def rel(p):
            assert _stack[-1] is p
            _stack.pop()
            p.release()

        _base_depth = [0]

        def stop_dump(src):
            """Truncated build: dump src, unwind pools opened within this pass."""
            for ct in range(CT):
                dma(out=out_d[:, ct, :], in_=src[:, ct, :].bitcast(f32))
            while len(_stack) > _base_depth[0]:
                rel(_stack[-1])
            raise _Stop

        pers = apool(name="pers", bufs=1)
        small = apool(name="small", bufs=1)
        scr = apool(name="scr", bufs=2)
        psb = apool(name="psb", bufs=3, space="PSUM")
        p_kv = apool(name="p_kv", bufs=1)

        # ---------- persistent loads ----------
        x_sb = pers.tile([128, CT, L], f32)
        h = pers.tile([128, CT, L], f32r)

        mask_sb = small.tile([128, CT, G], f32)
        dma(out=mask_sb, in_=mask_d)
        maskT_sb = small.tile([G, C], f32)
        dma(out=maskT_sb, in_=maskT_d)
        gn1g = small.tile([128, CT], f32); dma(out=gn1g, in_=gn1g_d)
        gn1b = small.tile([128, CT], f32); dma(out=gn1b, in_=gn1b_d)
        gn2g = small.tile([128, CT], f32); dma(out=gn2g, in_=gn2g_d)
        gn2b = small.tile([128, CT], f32); dma(out=gn2b, in_=gn2b_d)
        qkb = small.tile([128, 2 * CT], f32); dma(out=qkb, in_=qkb_d)
        vb_row = small.tile([1, C], f32); dma(out=vb_row, in_=vb_row_d)
        sapb = small.tile([128, CT], f32); dma(out=sapb, in_=sapb_d)
        qb = small.tile([128, CT], f32); dma(out=qb, in_=qb_d)
        kb = small.tile([128, CT], f32); dma(out=kb, in_=kb_d)
        vb2_row = small.tile([1, C], f32); dma(out=vb2_row, in_=vb2_row_d)
        capb = small.tile([128, CT], f32); dma(out=capb, in_=capb_d)
        b1 = small.tile([128, FT], f32); dma(out=b1, in_=b1_d)
        b2 = small.tile([128, CT], f32); dma(out=b2, in_=b2_d)
        smask = small.tile([128, 1], f32); dma(out=smask, in_=smask_d)

        vb_bc = small.tile([128, C], f32)
        nc.gpsimd.partition_broadcast(vb_bc, vb_row)
        vb2_bc = small.tile([128, C], f32)
        nc.gpsimd.partition_broadcast(vb2_bc, vb2_row)

        eps_t = small.tile([G, 1], f32)
        nc.vector.memset(eps_t, EPS)
        ones_t = small.tile([128, 1], f32)
        nc.vector.memset(ones_t, 1.0)
        zeros_t = small.tile([128, 1], f32)
        nc.vector.memset(zeros_t, 0.0)

        # cross-attention K/V live here across the whole pass
        k2 = p_kv.tile([128, CT, SP], bf16)
        v2_aug = p_kv.tile([128, NH * (HD + 1)], bf16)

        # ---------- phase 0: cross-attn K/V from context (before x arrives) ----------
        p_ctxw = apool(name="p_ctxw", bufs=1)
        ctxT = p_ctxw.tile([128, KTC, SP], bf16)
        dma(out=ctxT, in_=ctxT_d)
        kwT = p_ctxw.tile([128, KTC, C], bf16)
        dma(out=kwT, in_=kwT_d)
        vwT = p_ctxw.tile([128, KTC, C], bf16)
        dma(out=vwT, in_=vwT_d)

        for ct in range(CT):
            ps = psb.tile([128, SP], f32, tag="av", bufs=2, name=f"k2ps{ct}")
            for kt in range(KTC):
                nc.tensor.matmul(ps, kwT[:, kt, ct * 128:(ct + 1) * 128],
                                 ctxT[:, kt, :], start=(kt == 0), stop=(kt == KTC - 1))
            nc.vector.tensor_scalar_add(out=k2[:, ct, :], in0=ps, scalar1=kb[:, ct:ct + 1])
        nc.vector.tensor_copy(out=k2[:, :, S:SP],
                              in_=zeros_t.to_broadcast([128, CT, SP - S]))

        ps_v2 = psb.tile([128, C], f32, tag="ps", bufs=3)
        for kt in range(KTC):
            nc.tensor.matmul(ps_v2, ctxT[:, kt, :], vwT[:, kt, :],
                             start=(kt == 0), stop=(kt == KTC - 1))
        v2t = scr.tile([128, C], f32, tag="v2t")
        nc.vector.tensor_add(v2t, ps_v2, vb2_bc)
        nc.vector.tensor_scalar_mul(
            out=v2_aug.rearrange("p (h e) -> p h e", e=HD + 1)[:, :, 0:HD],
            in0=v2t.rearrange("p (h e) -> p h e", e=HD), scalar1=smask)
        nc.vector.tensor_copy(
            out=v2_aug.rearrange("p (h e) -> p h e", e=HD + 1)[:, :, HD:HD + 1],
            in_=smask.to_broadcast([128, NH, 1]))
        rel(p_ctxw)

        for ct in range(CT):
            dma(out=x_sb[:, ct, :], in_=x_d[:, ct, :])

        # ---------- GroupNorm helper ----------
        def groupnorm(src, dst, g_sb, b_sb, src_f32=False):
            cast = (lambda ap: ap) if src_f32 else (lambda ap: ap.bitcast(f32))
            stats = small.tile([128, CT, 2], f32, tag="gn_stats")
            for ct in range(CT):
                nc.vector.reduce_sum(out=stats[:, ct, 0:1], in_=cast(src[:, ct, :]), axis=AX)
            for ct in range(CT):
                sc = scr.tile([128, L], f32, tag="gn_scr")
                nc.scalar.activation(out=sc, in_=cast(src[:, ct, :]), func=Square,
                                     accum_out=stats[:, ct, 1:2])
            psg = psb.tile([G, 2], f32, tag="av", bufs=2)
            for ct in range(CT):
                nc.tensor.matmul(psg, mask_sb[:, ct, :], stats[:, ct, :],
                                 start=(ct == 0), stop=(ct == CT - 1))
            mv = small.tile([G, 2], f32, tag="gn_mv")
            nc.vector.tensor_scalar_mul(mv, psg, 1.0 / (16 * L))
            tmp = small.tile([G, 1], f32, tag="gn_tmp")
            nc.vector.tensor_mul(tmp, mv[:, 0:1], mv[:, 0:1])
            nc.vector.tensor_sub(mv[:, 1:2], mv[:, 1:2], tmp)
            sq = small.tile([G, 1], f32, tag="gn_sq")
            nc.scalar.activation(out=sq, in_=mv[:, 1:2], func=Sqrt, bias=eps_t)
            nc.vector.reciprocal(mv[:, 1:2], sq)
            ss = small.tile([128, CT, 2], f32, tag="gn_ss")
            for ct in range(CT):
                pc = psb.tile([128, 2], f32, tag="av", bufs=2)
                nc.tensor.matmul(pc, maskT_sb[:, ct * 128:(ct + 1) * 128], mv,
                                 start=True, stop=True)
                nc.vector.tensor_mul(ss[:, ct, 0:1], pc[:, 1:2], g_sb[:, ct:ct + 1])
                t2 = small.tile([128, 1], f32, tag="gn_t2")
                nc.vector.tensor_mul(t2, pc[:, 0:1], ss[:, ct, 0:1])
                nc.vector.tensor_sub(ss[:, ct, 1:2], b_sb[:, ct:ct + 1], t2)
            for ct in range(CT):
                nc.vector.tensor_scalar(
                    out=dst[:, ct, :], in0=cast(src[:, ct, :]),
                    scalar1=ss[:, ct, 0:1], scalar2=ss[:, ct, 1:2],
                    op0=mult, op1=add)

        _base_depth[0] = len(_stack)
        for _rep in range(repeat):
          try:
            # ---------- phase 1: the two GroupNorms ----------
            groupnorm(x_sb, h, gn1g, gn1b, src_f32=True)
            if stop_after == "gn1":
                stop_dump(h)

            p_ao = apool(name="p_ao", bufs=1)
            attn_out = p_ao.tile([128, CT, L], f32r)
            p_sap = apool(name="p_sap", bufs=1)
            sapT = p_sap.tile([128, CT, C], f32r)
            dma(out=sapT, in_=sapT_d)
            p_vaug = apool(name="p_vaug", bufs=1)
            v_aug = p_vaug.tile([128, LT, NH * (HD + 1)], bf16)
            p_qk = apool(name="p_qk", bufs=1)
            qk = p_qk.tile([128, 2 * CT, L], f32r)      # q tiles 0-3, k tiles 4-7

            p_hn = apool(name="p_hn", bufs=1)
            hn = p_hn.tile([128, CT, L], f32r)
            groupnorm(h, hn, gn2g, gn2b)

            # ---------- phase 2a: qkv ----------
            p_wqkv = apool(name="p_wqkv", bufs=1)
            qkvwT = p_wqkv.tile([128, CT, 3 * C], f32r)
            dma(out=qkvwT, in_=qkvwT_d)

            for mt in range(2 * CT):                    # q and k channel tiles
                ps = psb.tile([128, L], f32, tag="ps", bufs=3, name=f"qkps{mt}")
                for kt in range(CT):
                    for ch in range(NCH):
                        nc.tensor.matmul(ps[:, ch * CH:(ch + 1) * CH],
                                         qkvwT[:, kt, mt * 128:(mt + 1) * 128],
                                         hn[:, kt, ch * CH:(ch + 1) * CH],
                                         start=(kt == 0), stop=(kt == CT - 1))
                nc.vector.tensor_scalar_add(out=qk[:, mt, :], in0=ps,
                                            scalar1=qkb[:, mt:mt + 1])
            # v in transposed (sequence) layout, augmented with a ones column
            for mt in range(LT):
                ps = psb.tile([128, C], f32, tag="ps", bufs=3, name=f"vps{mt}")
                for kt in range(CT):
                    nc.tensor.matmul(ps, hn[:, kt, mt * 128:(mt + 1) * 128],
                                     qkvwT[:, kt, 2 * C:3 * C],
                                     start=(kt == 0), stop=(kt == CT - 1))
                nc.vector.tensor_add(
                    out=v_aug[:, mt, :].rearrange("p (h e) -> p h e", e=HD + 1)[:, :, 0:HD],
                    in0=ps.rearrange("p (h e) -> p h e", e=HD),
                    in1=vb_bc.rearrange("p (h e) -> p h e", e=HD))
            nc.vector.tensor_copy(
                out=v_aug.rearrange("p m (h e) -> p m h e", e=HD + 1)[:, :, :, HD:HD + 1],
                in_=ones_t.to_broadcast([128, LT, NH, 1]))

            rel(p_wqkv)
            rel(p_hn)
            if stop_after == "qkv":
                stop_dump(qk[:, 0:CT, :])

            # ---------- phase 2b: self-attention ----------
            p_pt = apool(name="p_pt", bufs=3)

            def sa_scores(hp):
                """S^T then exp for head pair (2hp, 2hp+1), row-group packed."""
                pts = [p_pt.tile([128, LT, L], bf16, tag="PT", bufs=3,
                                 name=f"pt{hp}_{i}") for i in range(2)]
                kt_ = 4 + hp
                for mt in range(LT):
                    pp = [psb.tile([128, L], f32, tag="ps", bufs=3,
                                   name=f"sps{hp}_{mt}_{i}") for i in range(2)]
                    for ch in range(NCH):
                        for i, po in ((0, 0), (1, 64)):
                            nc.tensor.matmul(
                                pp[i][:, ch * CH:(ch + 1) * CH],
                                qk[po:po + 64, kt_, mt * 128:(mt + 1) * 128],
                                qk[po:po + 64, hp, ch * CH:(ch + 1) * CH],
                                start=True, stop=True)
                    for i in range(2):
                        nc.scalar.activation(out=pts[i][:, mt, :], in_=pp[i],
                                             func=Exp, scale=SCALE)
                return pts

            def sa_av(hp, pts):
                for i in range(2):
                    hh = 2 * hp + i
                    for ch in range(NCH):
                        ps = psb.tile([HD + 1, CH], f32, tag="av", bufs=2,
                                      name=f"avps{hh}_{ch}")
                        for mt in range(LT):
                            nc.tensor.matmul(
                                ps, v_aug[:, mt, hh * (HD + 1):(hh + 1) * (HD + 1)],
                                pts[i][:, mt, ch * CH:(ch + 1) * CH],
                                start=(mt == 0), stop=(mt == LT - 1))
                        rec = scr.tile([1, CH], f32, tag="rec", bufs=4)
                        nc.vector.reciprocal(rec, ps[HD:HD + 1, :])
                        rb = scr.tile([HD, CH], f32, tag="recb", bufs=4)
                        nc.gpsimd.partition_broadcast(rb, rec)
                        nc.vector.tensor_mul(
                            out=attn_out[64 * i:64 * i + 64, hp, ch * CH:(ch + 1) * CH],
                            in0=ps[0:HD, :], in1=rb)

            prev = None
            for hp in range(CT):
                pts = sa_scores(hp)
                if prev is not None:
                    sa_av(*prev)
                prev = (hp, pts)
            sa_av(*prev)
            rel(p_pt)
            rel(p_qk)
            rel(p_vaug)

            # sa_proj + residual (h += proj(attn_out) + b)
            for ct in range(CT):
                for ch in range(NCH):
                    ps = psb.tile([128, CH], f32, tag="ps", bufs=3,
                                  name=f"sap{ct}_{ch}")
                    for kt in range(CT):
                        nc.tensor.matmul(ps, sapT[:, kt, ct * 128:(ct + 1) * 128],
                                         attn_out[:, kt, ch * CH:(ch + 1) * CH],
                                         start=(kt == 0), stop=(kt == CT - 1))
                    nc.vector.scalar_tensor_tensor(
                        out=h[:, ct, ch * CH:(ch + 1) * CH], in0=ps,
                        scalar=sapb[:, ct:ct + 1],
                        in1=h[:, ct, ch * CH:(ch + 1) * CH].bitcast(f32),
                        op0=add, op1=add)
            rel(p_sap)
            rel(p_ao)
            if stop_after == "sa":
                stop_dump(h)

            # ---------- phase 3: cross-attention ----------
            p_caa = apool(name="p_caa", bufs=1)
            q2 = p_caa.tile([128, CT, L], bf16)
            ca_out = p_caa.tile([128, CT, L], bf16)
            p_w1 = apool(name="p_w1", bufs=1)
            w1T = p_w1.tile([128, CT, FF], f32r)
            p_qcw = apool(name="p_qcw", bufs=1)
            qwT = p_qcw.tile([128, CT, C], f32r); dma(out=qwT, in_=qwT_d)
            capT = p_qcw.tile([128, CT, C], bf16); dma(out=capT, in_=capT_d)
            dma(out=w1T, in_=w1T_d)                     # prefetch during CA
            p_p2 = apool(name="p_p2", bufs=4)

            # q2 = q_w @ h
            for ct in range(CT):
                ps = psb.tile([128, L], f32, tag="ps", bufs=3, name=f"q2ps{ct}")
                for kt in range(CT):
                    for ch in range(NCH):
                        nc.tensor.matmul(ps[:, ch * CH:(ch + 1) * CH],
                                         qwT[:, kt, ct * 128:(ct + 1) * 128],
                                         h[:, kt, ch * CH:(ch + 1) * CH],
                                         start=(kt == 0), stop=(kt == CT - 1))
                nc.vector.tensor_scalar_add(out=q2[:, ct, :], in0=ps,
                                            scalar1=qb[:, ct:ct + 1])

            def ca_scores(hp):
                pp = [psb.tile([128, L], f32, tag="ps", bufs=3,
                               name=f"cps{hp}_{i}") for i in range(2)]
                for ch in range(NCH):
                    for i, po in ((0, 0), (1, 64)):
                        nc.tensor.matmul(pp[i][:, ch * CH:(ch + 1) * CH],
                                         k2[po:po + 64, hp, :],
                                         q2[po:po + 64, hp, ch * CH:(ch + 1) * CH],
                                         start=True, stop=True)
                p2s = []
                for i in range(2):
                    p2 = p_p2.tile([128, L], bf16, tag="P2", bufs=4, name=f"p2_{hp}_{i}")
                    nc.scalar.activation(out=p2, in_=pp[i], func=Exp, scale=SCALE)
                    p2s.append(p2)
                return p2s

            def ca_av(hp, p2s):
                for i in range(2):
                    hh = 2 * hp + i
                    for ch in range(NCH):
                        ps2 = psb.tile([HD + 1, CH], f32, tag="av", bufs=2,
                                       name=f"avp2_{hh}_{ch}")
                        nc.tensor.matmul(ps2, v2_aug[:, hh * (HD + 1):(hh + 1) * (HD + 1)],
                                         p2s[i][:, ch * CH:(ch + 1) * CH],
                                         start=True, stop=True)
                        rec = scr.tile([1, CH], f32, tag="rec", bufs=4)
                        nc.vector.reciprocal(rec, ps2[HD:HD + 1, :])
                        rb = scr.tile([HD, CH], f32, tag="recb", bufs=4)
                        nc.gpsimd.partition_broadcast(rb, rec)
                        nc.vector.tensor_mul(
                            out=ca_out[64 * i:64 * i + 64, hp, ch * CH:(ch + 1) * CH],
                            in0=ps2[0:HD, :], in1=rb)

            prev = None
            for hp in range(CT):
                p2s = ca_scores(hp)
                if prev is not None:
                    ca_av(*prev)
                prev = (hp, p2s)
            ca_av(*prev)

            # ca_proj + residual
            for ct in range(CT):
                for ch in range(NCH):
                    ps = psb.tile([128, CH], f32, tag="ps", bufs=3,
                                  name=f"cap{ct}_{ch}")
                    for kt in range(CT):
                        nc.tensor.matmul(ps, capT[:, kt, ct * 128:(ct + 1) * 128],
                                         ca_out[:, kt, ch * CH:(ch + 1) * CH],
                                         start=(kt == 0), stop=(kt == CT - 1))
                    nc.vector.scalar_tensor_tensor(
                        out=h[:, ct, ch * CH:(ch + 1) * CH], in0=ps,
                        scalar=capb[:, ct:ct + 1],
                        in1=h[:, ct, ch * CH:(ch + 1) * CH].bitcast(f32),
                        op0=add, op1=add)
            rel(p_p2)
            rel(p_qcw)
            if stop_after == "ca":
                stop_dump(h)

            # ---------- phase 4: FFN ----------
            p_w2 = apool(name="p_w2", bufs=1)
            w2T = p_w2.tile([128, FT, C], bf16)
            dma(out=w2T, in_=w2T_d)
            p_ff = apool(name="p_ff", bufs=1)
            ff1 = p_ff.tile([128, FT, L], bf16)
            p_of = apool(name="p_of", bufs=2)

            for ft in range(FT):
                ps = psb.tile([128, L], f32, tag="ps", bufs=3, name=f"f1ps{ft}")
                for kt in range(CT):
                    for ch in range(NCH):
                        nc.tensor.matmul(ps[:, ch * CH:(ch + 1) * CH],
                                         w1T[:, kt, ft * 128:(ft + 1) * 128],
                                         h[:, kt, ch * CH:(ch + 1) * CH],
                                         start=(kt == 0), stop=(kt == CT - 1))
                nc.scalar.activation(out=ff1[:, ft, :], in_=ps, func=Gelu,
                                     bias=b1[:, ft:ft + 1], scale=1.0)
            for ct in range(CT):
                for ch in range(NCH):
                    ps = psb.tile([128, CH], f32, tag="ps", bufs=3,
                                  name=f"f2ps{ct}_{ch}")
                    for kt in range(FT):
                        nc.tensor.matmul(ps, w2T[:, kt, ct * 128:(ct + 1) * 128],
                                         ff1[:, kt, ch * CH:(ch + 1) * CH],
                                         start=(kt == 0), stop=(kt == FT - 1))
                    nc.vector.scalar_tensor_tensor(
                        out=h[:, ct, ch * CH:(ch + 1) * CH], in0=ps,
                        scalar=b2[:, ct:ct + 1],
                        in1=h[:, ct, ch * CH:(ch + 1) * CH].bitcast(f32),
                        op0=add, op1=add)
                    of = p_of.tile([128, CH], f32, tag="of")
                    nc.vector.tensor_add(of, h[:, ct, ch * CH:(ch + 1) * CH].bitcast(f32),
                                         x_sb[:, ct, ch * CH:(ch + 1) * CH])
                    dma(out=out_d[:, ct, ch * CH:(ch + 1) * CH], in_=of)

            for p in (p_of, p_ff, p_w2, p_w1, p_caa):
                rel(p)
          except _Stop:
            pass
        for p in (p_kv, psb, scr, small, pers):
            rel(p)

    nc.compile()
    return nc


def _tileK(wT, kt, dt=np.float32):
    """[K, F] -> [128, kt, F] partition-major layout."""
    K, F = wT.shape
    return np.ascontiguousarray(
        wT.reshape(kt, 128, F).transpose(1, 0, 2)).astype(dt)


def _conv(b):
    """[n] -> [128, n//128] conv-layout bias."""
    return np.ascontiguousarray(np.asarray(b, np.float32).reshape(-1, 128).T)


def prepare_in_maps(inputs):
    import ml_dtypes
    bf = ml_dtypes.bfloat16
    f = lambda a: np.asarray(a, np.float32)
    x = f(inputs["x"]); ctx = f(inputs["context"])
    shared = {
        "qkv_wT": _tileK(f(inputs["qkv_w"]).T, CT),
        "sa_proj_wT": _tileK(f(inputs["sa_proj_w"]).T, CT),
        "q_wT": _tileK(f(inputs["q_w"]).T, CT),
        "k_wT": _tileK(f(inputs["k_w"]).T, KTC, bf),
        "v_wT": _tileK(f(inputs["v_w"]).T, KTC, bf),
        "ca_proj_wT": _tileK(f(inputs["ca_proj_w"]).T, CT, bf),
        "w1T": _tileK(f(inputs["w1"]).T, CT),
        "w2T": _tileK(f(inputs["w2"]).T, FT, bf),
        "gn1g": _conv(inputs["gn_in_g"]), "gn1b": _conv(inputs["gn_in_b"]),
        "gn2g": _conv(inputs["sa_gn_g"]), "gn2b": _conv(inputs["sa_gn_b"]),
        "qkb": _conv(f(inputs["qkv_b"])[:2 * C]),
        "vb_row": f(inputs["qkv_b"])[2 * C:].reshape(1, C).copy(),
        "sapb": _conv(inputs["sa_proj_b"]),
        "qb": _conv(inputs["q_b"]), "kb": _conv(inputs["k_b"]),
        "vb2_row": f(inputs["v_b"]).reshape(1, C).copy(),
        "capb": _conv(inputs["ca_proj_b"]),
        "b1": _conv(inputs["b1"]), "b2": _conv(inputs["b2"]),
    }
    cidx = np.arange(C) // 16
    mask = (cidx[:, None] == np.arange(G)[None, :]).astype(np.float32)  # [C, G]
    shared["gn_mask"] = np.ascontiguousarray(
        mask.reshape(CT, 128, G).transpose(1, 0, 2))
    shared["gn_maskT"] = np.ascontiguousarray(mask.T)
    shared["smask"] = (np.arange(SP) < S).astype(np.float32).reshape(SP, 1)

    in_maps = []
    for b in range(B):
        xb = np.ascontiguousarray(
            x[b].reshape(C, L).reshape(CT, 128, L).transpose(1, 0, 2))
        ctxT = np.zeros((CTX, SP), np.float32)
        ctxT[:, :S] = ctx[b].T
        ctxTb = np.ascontiguousarray(
            ctxT.reshape(KTC, 128, SP).transpose(1, 0, 2)).astype(bf)
        in_maps.append({"x": xb, "ctxT": ctxTb, **shared})
    return in_maps


def kernel(**inputs):
    from concourse.bass_utils import run_bass_kernel_spmd
    if "nc" not in _CACHE:
        _CACHE["nc"] = _build()
    nc = _CACHE["nc"]
    in_maps = prepare_in_maps(inputs)
    res = run_bass_kernel_spmd(nc, in_maps, core_ids=list(range(B)))
    out = np.stack([
        np.ascontiguousarray(res.results[b]["out"].transpose(1, 0, 2)).reshape(C, H, W)
        for b in range(B)])
    return out.astype(np.float32)


# revision 33
# speedup vs baseline: 13701.0213x; 13701.0213x over previous
"""Trainium2 Bass kernel for nn_AttentionBlock (GN + self-attn + cross-attn + FFN).

Sharding: data-parallel over batch B=8 -> one batch element per NeuronCore.
Per-core layout: activations as [C(partitions), L(free)] "conv" layout.
Attention computed with transposed scores S^T[m, l]; softmax sums come from an
augmented-V matmul (extra ones column -> Z lands in psum partition 64), so no
cross-partition reductions or transposes are needed. Row-softmax max-subtraction
is skipped (logits are provably < 2 for this block's scale).
Matmuls run in float32r (rounded fp32, full PE speed at N>=256); attention
probabilities / V / cross-attention / FFN-hidden run in bf16.
"""
import sys

for _p in ("/opt/trn_rl_repo", "/root/.axon_site/_ro/trn_rl_repo"):
    if _p not in sys.path:
        sys.path.append(_p)

import numpy as np

# ---- problem constants (hardcoded per contract) ----
B, C, H, W = 8, 512, 32, 32
L = H * W                       # 1024
NH, HD = 8, 64
CT = C // 128                   # 4 channel tiles
LT = L // 128                   # 8 l/m tiles
NCH = 2                         # l chunks of 512
CH = L // NCH                   # 512
CTX = 768
S = 77
SP = 128                        # padded context tokens
KTC = CTX // 128                # 6
FF = 4 * C                      # 2048
FT = FF // 128                  # 16
G = 32                          # groups
EPS = 1e-5
SCALE = HD ** -0.5

_CACHE = {}


def _build(gelu_identity=False, stop_after=None, repeat=1, gn2_skip=False):
    import concourse.mybir as mybir
    import concourse.tile as tile
    from concourse import bacc

    f32 = mybir.dt.float32
    f32r = mybir.dt.float32r
    bf16 = mybir.dt.bfloat16
    Exp = mybir.ActivationFunctionType.Exp
    Gelu = (mybir.ActivationFunctionType.Identity if gelu_identity
            else mybir.ActivationFunctionType.Gelu)
    Sqrt = mybir.ActivationFunctionType.Sqrt
    Square = mybir.ActivationFunctionType.Square
    add = mybir.AluOpType.add
    mult = mybir.AluOpType.mult
    AX = mybir.AxisListType.X

    nc = bacc.Bacc("TRN2", target_bir_lowering=False, debug=False, num_devices=8)

    def din(name, shape, dt=f32r):
        return nc.dram_tensor(name, shape, dt, kind="ExternalInput").ap()

    x_d = din("x", [128, CT, L], f32)
    ctxT_d = din("ctxT", [128, KTC, SP], bf16)
    qkvwT_d = din("qkv_wT", [128, CT, 3 * C])
    sapT_d = din("sa_proj_wT", [128, CT, C], bf16)
    qwT_d = din("q_wT", [128, CT, C])
    kwT_d = din("k_wT", [128, KTC, C], bf16)
    vwT_d = din("v_wT", [128, KTC, C], bf16)
    capT_d = din("ca_proj_wT", [128, CT, C], bf16)
    w1T_d = din("w1T", [128, CT, FF])
    w2T_d = din("w2T", [128, FT, C], bf16)
    mask_d = din("gn_mask", [128, CT, G], f32)
    maskT_d = din("gn_maskT", [G, C], f32)
    gn1g_d = din("gn1g", [128, CT], f32)
    gn1b_d = din("gn1b", [128, CT], f32)
    gn2g_d = din("gn2g", [128, CT], f32)
    gn2b_d = din("gn2b", [128, CT], f32)
    qkb_d = din("qkb", [128, 2 * CT], f32)     # qkv_b for q,k in conv layout
    vb_row_d = din("vb_row", [1, C], f32)      # qkv_b v-part as a row
    sapb_d = din("sapb", [128, CT], f32)
    qb_d = din("qb", [128, CT], f32)
    kb_d = din("kb", [128, CT], f32)
    vb2_row_d = din("vb2_row", [1, C], f32)
    capb_d = din("capb", [128, CT], f32)
    b1_d = din("b1", [128, FT], f32)
    b2_d = din("b2", [128, CT], f32)
    smask_d = din("smask", [128, 1], f32)      # context token validity column

    out_d = nc.dram_tensor("out", [128, CT, L], f32, kind="ExternalOutput").ap()

    dma = nc.sync.dma_start

    class _Stop(Exception):
        pass

    with tile.TileContext(nc) as tc:
        _stack = []

        def apool(**kw):
            p = tc.alloc_tile_pool(**kw)
            _stack.append(p)
            return p

        def rel(p):
            assert _stack[-1] is p
            _stack.pop()
            p.release()

        _base_depth = [0]

        def stop_dump(src):
            """Truncated build: dump src, unwind pools opened within this pass."""
            for ct in range(CT):
                dma(out=out_d[:, ct, :], in_=src[:, ct, :].bitcast(f32))
            while len(_stack) > _base_depth[0]:
                rel(_stack[-1])
            raise _Stop

        pers = apool(name="pers", bufs=1)
        small = apool(name="small", bufs=1)
        scr = apool(name="scr", bufs=2)
        psb = apool(name="psb", bufs=3, space="PSUM")
        p_kv = apool(name="p_kv", bufs=1)

        # ---------- persistent loads ----------
        x_sb = pers.tile([128, CT, L], f32)
        h = pers.tile([128, CT, L], f32r)

        mask_sb = small.tile([128, CT, G], f32)
        dma(out=mask_sb, in_=mask_d)
        maskT_sb = small.tile([G, C], f32)
        dma(out=maskT_sb, in_=maskT_d)
        gn1g = small.tile([128, CT], f32); dma(out=gn1g, in_=gn1g_d)
        gn1b = small.tile([128, CT], f32); dma(out=gn1b, in_=gn1b_d)
        gn2g = small.tile([128, CT], f32); dma(out=gn2g, in_=gn2g_d)
        gn2b = small.tile([128, CT], f32); dma(out=gn2b, in_=gn2b_d)
        qkb = small.tile([128, 2 * CT], f32); dma(out=qkb, in_=qkb_d)
        vb_row = small.tile([1, C], f32); dma(out=vb_row, in_=vb_row_d)
        sapb = small.tile([128, CT], f32); dma(out=sapb, in_=sapb_d)
        qb = small.tile([128, CT], f32); dma(out=qb, in_=qb_d)
        kb = small.tile([128, CT], f32); dma(out=kb, in_=kb_d)
        vb2_row = small.tile([1, C], f32); dma(out=vb2_row, in_=vb2_row_d)
        capb = small.tile([128, CT], f32); dma(out=capb, in_=capb_d)
        b1 = small.tile([128, FT], f32); dma(out=b1, in_=b1_d)
        b2 = small.tile([128, CT], f32); dma(out=b2, in_=b2_d)
        smask = small.tile([128, 1], f32); dma(out=smask, in_=smask_d)

        vb_bc = small.tile([128, C], f32)
        nc.gpsimd.partition_broadcast(vb_bc, vb_row)
        vb2_bc = small.tile([128, C], f32)
        nc.gpsimd.partition_broadcast(vb2_bc, vb2_row)

        eps_t = small.tile([G, 1], f32)
        nc.vector.memset(eps_t, EPS)
        ones_t = small.tile([128, 1], f32)
        nc.vector.memset(ones_t, 1.0)
        zeros_t = small.tile([128, 1], f32)
        nc.vector.memset(zeros_t, 0.0)

        # cross-attention K/V live here across the whole pass
        k2 = p_kv.tile([128, CT, SP], bf16)
        v2_aug = p_kv.tile([128, NH * (HD + 1)], bf16)

        # ---------- phase 0: cross-attn K/V from context (before x arrives) ----------
        p_ctxw = apool(name="p_ctxw", bufs=1)
        ctxT = p_ctxw.tile([128, KTC, SP], bf16)
        dma(out=ctxT, in_=ctxT_d)
        kwT = p_ctxw.tile([128, KTC, C], bf16)
        dma(out=kwT, in_=kwT_d)
        vwT = p_ctxw.tile([128, KTC, C], bf16)
        dma(out=vwT, in_=vwT_d)

        for ct in range(CT):
            ps = psb.tile([128, SP], f32, tag="av", bufs=2, name=f"k2ps{ct}")
            for kt in range(KTC):
                nc.tensor.matmul(ps, kwT[:, kt, ct * 128:(ct + 1) * 128],
                                 ctxT[:, kt, :], start=(kt == 0), stop=(kt == KTC - 1))
            nc.vector.tensor_scalar_add(out=k2[:, ct, :], in0=ps, scalar1=kb[:, ct:ct + 1])
        nc.vector.tensor_copy(out=k2[:, :, S:SP],
                              in_=zeros_t.to_broadcast([128, CT, SP - S]))

        ps_v2 = psb.tile([128, C], f32, tag="ps", bufs=3)
        for kt in range(KTC):
            nc.tensor.matmul(ps_v2, ctxT[:, kt, :], vwT[:, kt, :],
                             start=(kt == 0), stop=(kt == KTC - 1))
        v2t = scr.tile([128, C], f32, tag="v2t")
        nc.vector.tensor_add(v2t, ps_v2, vb2_bc)
        nc.vector.tensor_scalar_mul(
            out=v2_aug.rearrange("p (h e) -> p h e", e=HD + 1)[:, :, 0:HD],
            in0=v2t.rearrange("p (h e) -> p h e", e=HD), scalar1=smask)
        nc.vector.tensor_copy(
            out=v2_aug.rearrange("p (h e) -> p h e", e=HD + 1)[:, :, HD:HD + 1],
            in_=smask.to_broadcast([128, NH, 1]))
        rel(p_ctxw)

        for ct in range(CT):
            dma(out=x_sb[:, ct, :], in_=x_d[:, ct, :])

        # ---------- GroupNorm helper ----------
        def groupnorm(src, dst, g_sb, b_sb, src_f32=False):
            cast = (lambda ap: ap) if src_f32 else (lambda ap: ap.bitcast(f32))
            stats = small.tile([128, CT, 2], f32, tag="gn_stats")
            for ct in range(CT):
                nc.vector.reduce_sum(out=stats[:, ct, 0:1], in_=cast(src[:, ct, :]), axis=AX)
            for ct in range(CT):
                sc = scr.tile([128, L], f32, tag="gn_scr", bufs=1)
                nc.scalar.activation(out=sc, in_=cast(src[:, ct, :]), func=Square,
                                     accum_out=stats[:, ct, 1:2])
            psg = psb.tile([G, 2], f32, tag="av", bufs=2)
            for ct in range(CT):
                nc.tensor.matmul(psg, mask_sb[:, ct, :], stats[:, ct, :],
                                 start=(ct == 0), stop=(ct == CT - 1))
            mv = small.tile([G, 2], f32, tag="gn_mv")
            nc.vector.tensor_scalar_mul(mv, psg, 1.0 / (16 * L))
            tmp = small.tile([G, 1], f32, tag="gn_tmp")
            nc.vector.tensor_mul(tmp, mv[:, 0:1], mv[:, 0:1])
            nc.vector.tensor_sub(mv[:, 1:2], mv[:, 1:2], tmp)
            sq = small.tile([G, 1], f32, tag="gn_sq")
            nc.scalar.activation(out=sq, in_=mv[:, 1:2], func=Sqrt, bias=eps_t)
            nc.vector.reciprocal(mv[:, 1:2], sq)
            ss = small.tile([128, CT, 2], f32, tag="gn_ss")
            for ct in range(CT):
                pc = psb.tile([128, 2], f32, tag="av", bufs=2)
                nc.tensor.matmul(pc, maskT_sb[:, ct * 128:(ct + 1) * 128], mv,
                                 start=True, stop=True)
                nc.vector.tensor_mul(ss[:, ct, 0:1], pc[:, 1:2], g_sb[:, ct:ct + 1])
                t2 = small.tile([128, 1], f32, tag="gn_t2")
                nc.vector.tensor_mul(t2, pc[:, 0:1], ss[:, ct, 0:1])
                nc.vector.tensor_sub(ss[:, ct, 1:2], b_sb[:, ct:ct + 1], t2)
            for ct in range(CT):
                nc.vector.tensor_scalar(
                    out=dst[:, ct, :], in0=cast(src[:, ct, :]),
                    scalar1=ss[:, ct, 0:1], scalar2=ss[:, ct, 1:2],
                    op0=mult, op1=add)

        _base_depth[0] = len(_stack)
        for _rep in range(repeat):
          try:
            # ---------- phase 1: the two GroupNorms ----------
            groupnorm(x_sb, h, gn1g, gn1b, src_f32=True)
            if stop_after == "gn1":
                stop_dump(h)

            p_ao = apool(name="p_ao", bufs=1)
            attn_out = p_ao.tile([128, CT, L], bf16)
            p_sap = apool(name="p_sap", bufs=1)
            sapT = p_sap.tile([128, CT, C], bf16)
            dma(out=sapT, in_=sapT_d)
            p_vaug = apool(name="p_vaug", bufs=1)
            v_aug = p_vaug.tile([128, LT, NH * (HD + 1)], bf16)
            p_qk = apool(name="p_qk", bufs=1)
            qk = p_qk.tile([128, 2 * CT, L], bf16)      # q tiles 0-3, k tiles 4-7

            if gn2_skip:
                hn = h          # sa_gn is identity and gn_in output is normalized
            else:
                p_hn = apool(name="p_hn", bufs=1)
                hn = p_hn.tile([128, CT, L], f32r)
                groupnorm(h, hn, gn2g, gn2b)

            # ---------- phase 2a: qkv ----------
            p_wqkv = apool(name="p_wqkv", bufs=1)
            qkvwT = p_wqkv.tile([128, CT, 3 * C], f32r)
            dma(out=qkvwT, in_=qkvwT_d)

            p_pt = apool(name="p_pt", bufs=3)

            def sa_scores(hp):
                """S^T then exp for head pair (2hp, 2hp+1), row-group packed."""
                pts = [p_pt.tile([128, LT, L], bf16, tag="PT", bufs=3,
                                 name=f"pt{hp}_{i}") for i in range(2)]
                kt_ = 4 + hp
                for mt in range(LT):
                    pp = [psb.tile([128, L], f32, tag="ps", bufs=3,
                                   name=f"sps{hp}_{mt}_{i}") for i in range(2)]
                    for ch in range(NCH):
                        for i, po in ((0, 0), (1, 64)):
                            nc.tensor.matmul(
                                pp[i][:, ch * CH:(ch + 1) * CH],
                                qk[po:po + 64, kt_, mt * 128:(mt + 1) * 128],
                                qk[po:po + 64, hp, ch * CH:(ch + 1) * CH],
                                start=True, stop=True)
                    for i in range(2):
                        nc.scalar.activation(out=pts[i][:, mt, :], in_=pp[i],
                                             func=Exp, scale=SCALE)
                return pts

            def qkv_group(mt):
                ps = psb.tile([128, L], f32, tag="ps", bufs=3, name=f"qkps{mt}")
                for kt in range(CT):
                    for ch in range(NCH):
                        nc.tensor.matmul(ps[:, ch * CH:(ch + 1) * CH],
                                         qkvwT[:, kt, mt * 128:(mt + 1) * 128],
                                         hn[:, kt, ch * CH:(ch + 1) * CH],
                                         start=(kt == 0), stop=(kt == CT - 1))
                nc.vector.tensor_scalar_add(out=qk[:, mt, :], in0=ps,
                                            scalar1=qkb[:, mt:mt + 1])

            for hp in range(CT):                        # q/k paired per head pair
                qkv_group(hp)
                qkv_group(4 + hp)
            # v in transposed (sequence) layout, augmented with a ones column
            for mt in range(LT):
                ps = psb.tile([128, C], f32, tag="ps", bufs=3, name=f"vps{mt}")
                for kt in range(CT):
                    nc.tensor.matmul(ps, hn[:, kt, mt * 128:(mt + 1) * 128],
                                     qkvwT[:, kt, 2 * C:3 * C],
                                     start=(kt == 0), stop=(kt == CT - 1))
                nc.vector.tensor_add(
                    out=v_aug[:, mt, :].rearrange("p (h e) -> p h e", e=HD + 1)[:, :, 0:HD],
                    in0=ps.rearrange("p (h e) -> p h e", e=HD),
                    in1=vb_bc.rearrange("p (h e) -> p h e", e=HD))
            nc.vector.tensor_copy(
                out=v_aug.rearrange("p m (h e) -> p m h e", e=HD + 1)[:, :, :, HD:HD + 1],
                in_=ones_t.to_broadcast([128, LT, NH, 1]))
            pts0 = sa_scores(0)

            if stop_after == "qkv":
                stop_dump(qk[:, 0:CT, :])

            # ---------- phase 2b: self-attention ----------
            def sa_av(hp, pts):
                for i in range(2):
                    hh = 2 * hp + i
                    for ch in range(NCH):
                        ps = psb.tile([HD + 1, CH], f32, tag="av", bufs=2,
                                      name=f"avps{hh}_{ch}")
                        for mt in range(LT):
                            nc.tensor.matmul(
                                ps, v_aug[:, mt, hh * (HD + 1):(hh + 1) * (HD + 1)],
                                pts[i][:, mt, ch * CH:(ch + 1) * CH],
                                start=(mt == 0), stop=(mt == LT - 1))
                        rec = scr.tile([1, CH], f32, tag="rec", bufs=6)
                        nc.vector.reciprocal(rec, ps[HD:HD + 1, :])
                        rb = scr.tile([HD, CH], f32, tag="recb", bufs=6)
                        nc.gpsimd.partition_broadcast(rb, rec)
                        nc.vector.tensor_mul(
                            out=attn_out[64 * i:64 * i + 64, hp, ch * CH:(ch + 1) * CH],
                            in0=ps[0:HD, :], in1=rb)

            prev = (0, pts0)
            for hp in range(1, CT):
                pts = sa_scores(hp)
                sa_av(*prev)
                prev = (hp, pts)
            sa_av(*prev)
            rel(p_pt)
            rel(p_wqkv)
            if not gn2_skip:
                rel(p_hn)
            rel(p_qk)
            rel(p_vaug)

            # sa_proj + residual (h += proj(attn_out) + b)
            for ct in range(CT):
                for ch in range(NCH):
                    ps = psb.tile([128, CH], f32, tag="ps", bufs=3,
                                  name=f"sap{ct}_{ch}")
                    for kt in range(CT):
                        nc.tensor.matmul(ps, sapT[:, kt, ct * 128:(ct + 1) * 128],
                                         attn_out[:, kt, ch * CH:(ch + 1) * CH],
                                         start=(kt == 0), stop=(kt == CT - 1))
                    nc.vector.scalar_tensor_tensor(
                        out=h[:, ct, ch * CH:(ch + 1) * CH], in0=ps,
                        scalar=sapb[:, ct:ct + 1],
                        in1=h[:, ct, ch * CH:(ch + 1) * CH].bitcast(f32),
                        op0=add, op1=add)
            rel(p_sap)
            rel(p_ao)
            if stop_after == "sa":
                stop_dump(h)

            # ---------- phase 3: cross-attention ----------
            p_caa = apool(name="p_caa", bufs=1)
            q2 = p_caa.tile([128, CT, L], bf16)
            ca_out = p_caa.tile([128, CT, L], bf16)
            p_w1 = apool(name="p_w1", bufs=1)
            w1T = p_w1.tile([128, CT, FF], f32r)
            p_qcw = apool(name="p_qcw", bufs=1)
            qwT = p_qcw.tile([128, CT, C], f32r); dma(out=qwT, in_=qwT_d)
            capT = p_qcw.tile([128, CT, C], bf16); dma(out=capT, in_=capT_d)
            dma(out=w1T, in_=w1T_d)                     # prefetch during CA
            p_p2 = apool(name="p_p2", bufs=4)

            # q2 = q_w @ h (interleaved with scores below)
            def q2_group(ct):
                ps = psb.tile([128, L], f32, tag="ps", bufs=3, name=f"q2ps{ct}")
                for kt in range(CT):
                    for ch in range(NCH):
                        nc.tensor.matmul(ps[:, ch * CH:(ch + 1) * CH],
                                         qwT[:, kt, ct * 128:(ct + 1) * 128],
                                         h[:, kt, ch * CH:(ch + 1) * CH],
                                         start=(kt == 0), stop=(kt == CT - 1))
                nc.vector.tensor_scalar_add(out=q2[:, ct, :], in0=ps,
                                            scalar1=qb[:, ct:ct + 1])

            def ca_scores(hp):
                pp = [psb.tile([128, L], f32, tag="ps", bufs=3,
                               name=f"cps{hp}_{i}") for i in range(2)]
                for ch in range(NCH):
                    for i, po in ((0, 0), (1, 64)):
                        nc.tensor.matmul(pp[i][:, ch * CH:(ch + 1) * CH],
                                         k2[po:po + 64, hp, :],
                                         q2[po:po + 64, hp, ch * CH:(ch + 1) * CH],
                                         start=True, stop=True)
                p2s = []
                for i in range(2):
                    p2 = p_p2.tile([128, L], bf16, tag="P2", bufs=8, name=f"p2_{hp}_{i}")
                    nc.scalar.activation(out=p2, in_=pp[i], func=Exp, scale=SCALE)
                    p2s.append(p2)
                return p2s

            def ca_av(hp, p2s):
                for i in range(2):
                    hh = 2 * hp + i
                    for ch in range(NCH):
                        ps2 = psb.tile([HD + 1, CH], f32, tag="av", bufs=2,
                                       name=f"avp2_{hh}_{ch}")
                        nc.tensor.matmul(ps2, v2_aug[:, hh * (HD + 1):(hh + 1) * (HD + 1)],
                                         p2s[i][:, ch * CH:(ch + 1) * CH],
                                         start=True, stop=True)
                        rec = scr.tile([1, CH], f32, tag="rec", bufs=6)
                        nc.vector.reciprocal(rec, ps2[HD:HD + 1, :])
                        rb = scr.tile([HD, CH], f32, tag="recb", bufs=6)
                        nc.gpsimd.partition_broadcast(rb, rec)
                        nc.vector.tensor_mul(
                            out=ca_out[64 * i:64 * i + 64, hp, ch * CH:(ch + 1) * CH],
                            in0=ps2[0:HD, :], in1=rb)

            all_p2 = []
            for hp in range(CT):
                q2_group(hp)
                all_p2.append(ca_scores(hp))
            for hp in range(CT):
                ca_av(hp, all_p2[hp])

            # ca_proj + residual
            for ct in range(CT):
                for ch in range(NCH):
                    ps = psb.tile([128, CH], f32, tag="ps", bufs=3,
                                  name=f"cap{ct}_{ch}")
                    for kt in range(CT):
                        nc.tensor.matmul(ps, capT[:, kt, ct * 128:(ct + 1) * 128],
                                         ca_out[:, kt, ch * CH:(ch + 1) * CH],
                                         start=(kt == 0), stop=(kt == CT - 1))
                    nc.vector.scalar_tensor_tensor(
                        out=h[:, ct, ch * CH:(ch + 1) * CH], in0=ps,
                        scalar=capb[:, ct:ct + 1],
                        in1=h[:, ct, ch * CH:(ch + 1) * CH].bitcast(f32),
                        op0=add, op1=add)
            rel(p_p2)
            rel(p_qcw)
            if stop_after == "ca":
                stop_dump(h)
            for ct in range(CT):
                nc.vector.tensor_add(x_sb[:, ct, :], h[:, ct, :].bitcast(f32),
                                     x_sb[:, ct, :])

            # ---------- phase 4: FFN ----------
            p_w2 = apool(name="p_w2", bufs=1)
            w2T = p_w2.tile([128, FT, C], bf16)
            dma(out=w2T, in_=w2T_d)
            p_ff = apool(name="p_ff", bufs=1)
            ff1 = p_ff.tile([128, FT, L], bf16)
            p_of = apool(name="p_of", bufs=2)

            for ft in range(FT):
                ps = psb.tile([128, L], f32, tag="ps", bufs=3, name=f"f1ps{ft}")
                for kt in range(CT):
                    for ch in range(NCH):
                        nc.tensor.matmul(ps[:, ch * CH:(ch + 1) * CH],
                                         w1T[:, kt, ft * 128:(ft + 1) * 128],
                                         h[:, kt, ch * CH:(ch + 1) * CH],
                                         start=(kt == 0), stop=(kt == CT - 1))
                nc.scalar.activation(out=ff1[:, ft, :], in_=ps, func=Gelu,
                                     bias=b1[:, ft:ft + 1], scale=1.0)
            for ct in range(CT):
                for ch in range(NCH):
                    ps = psb.tile([128, CH], f32, tag="av", bufs=2,
                                  name=f"f2ps{ct}_{ch}")
                    for kt in range(FT):
                        nc.tensor.matmul(ps, w2T[:, kt, ct * 128:(ct + 1) * 128],
                                         ff1[:, kt, ch * CH:(ch + 1) * CH],
                                         start=(kt == 0), stop=(kt == FT - 1))
                    of = p_of.tile([128, CH], f32, tag="of")
                    nc.vector.scalar_tensor_tensor(
                        out=of, in0=ps, scalar=b2[:, ct:ct + 1],
                        in1=x_sb[:, ct, ch * CH:(ch + 1) * CH],
                        op0=add, op1=add)
                    dma(out=out_d[:, ct, ch * CH:(ch + 1) * CH], in_=of)

            for p in (p_of, p_ff, p_w2, p_w1, p_caa):
                rel(p)
          except _Stop:
            pass
        for p in (p_kv, psb, scr, small, pers):
            rel(p)

    nc.compile()
    return nc


def _tileK(wT, kt, dt=np.float32):
    """[K, F] -> [128, kt, F] partition-major layout."""
    K, F = wT.shape
    return np.ascontiguousarray(
        wT.reshape(kt, 128, F).transpose(1, 0, 2)).astype(dt)


def _conv(b):
    """[n] -> [128, n//128] conv-layout bias."""
    return np.ascontiguousarray(np.asarray(b, np.float32).reshape(-1, 128).T)


def prepare_in_maps(inputs):
    import ml_dtypes
    bf = ml_dtypes.bfloat16
    f = lambda a: np.asarray(a, np.float32)
    x = f(inputs["x"]); ctx = f(inputs["context"])
    shared = {
        "qkv_wT": _tileK(f(inputs["qkv_w"]).T, CT),
        "sa_proj_wT": _tileK(f(inputs["sa_proj_w"]).T, CT, bf),
        "q_wT": _tileK(f(inputs["q_w"]).T, CT),
        "k_wT": _tileK(f(inputs["k_w"]).T, KTC, bf),
        "v_wT": _tileK(f(inputs["v_w"]).T, KTC, bf),
        "ca_proj_wT": _tileK(f(inputs["ca_proj_w"]).T, CT, bf),
        "w1T": _tileK(f(inputs["w1"]).T, CT),
        "w2T": _tileK(f(inputs["w2"]).T, FT, bf),
        "gn1g": _conv(inputs["gn_in_g"]), "gn1b": _conv(inputs["gn_in_b"]),
        "gn2g": _conv(inputs["sa_gn_g"]), "gn2b": _conv(inputs["sa_gn_b"]),
        "qkb": _conv(f(inputs["qkv_b"])[:2 * C]),
        "vb_row": f(inputs["qkv_b"])[2 * C:].reshape(1, C).copy(),
        "sapb": _conv(inputs["sa_proj_b"]),
        "qb": _conv(inputs["q_b"]), "kb": _conv(inputs["k_b"]),
        "vb2_row": f(inputs["v_b"]).reshape(1, C).copy(),
        "capb": _conv(inputs["ca_proj_b"]),
        "b1": _conv(inputs["b1"]), "b2": _conv(inputs["b2"]),
    }
    cidx = np.arange(C) // 16
    mask = (cidx[:, None] == np.arange(G)[None, :]).astype(np.float32)  # [C, G]
    shared["gn_mask"] = np.ascontiguousarray(
        mask.reshape(CT, 128, G).transpose(1, 0, 2))
    shared["gn_maskT"] = np.ascontiguousarray(mask.T)
    shared["smask"] = (np.arange(SP) < S).astype(np.float32).reshape(SP, 1)

    in_maps = []
    for b in range(B):
        xb = np.ascontiguousarray(
            x[b].reshape(C, L).reshape(CT, 128, L).transpose(1, 0, 2))
        ctxT = np.zeros((CTX, SP), np.float32)
        ctxT[:, :S] = ctx[b].T
        ctxTb = np.ascontiguousarray(
            ctxT.reshape(KTC, 128, SP).transpose(1, 0, 2)).astype(bf)
        in_maps.append({"x": xb, "ctxT": ctxTb, **shared})
    return in_maps


def kernel(**inputs):
    from concourse.bass_utils import run_bass_kernel_spmd
    if "nc" not in _CACHE:
        _CACHE["nc"] = _build()
    nc = _CACHE["nc"]
    in_maps = prepare_in_maps(inputs)
    res = run_bass_kernel_spmd(nc, in_maps, core_ids=list(range(B)))
    out = np.stack([
        np.ascontiguousarray(res.results[b]["out"].transpose(1, 0, 2)).reshape(C, H, W)
        for b in range(B)])
    return out.astype(np.float32)
